# revision 17
# baseline (speedup 1.0000x reference)
"""Trainium2 Bass kernel for AdaptiveModalitySelectionSystem (moe_routing).

Data-parallel over batch B=4096 across 8 NeuronCores (B_local=512 each).
Per core:
  - Router MLP computed in transposed layout: hT = (ctx @ W1 + b1)^T [RH, 512]
    via W1 as the stationary operand, LayerNorm via PE column-sum matmuls,
    W2/W3 GEMMs stay transposed down to logits^T [K, 512]; small per-b-tile
    transposes bring logits back to [b, K] for the gumbel-sigmoid + forced
    top-2 mask pipeline.
  - coef[b,k] = mask*(mask>0.5)*softmax(fusion_w)[k]; top-2 computed on
    logits (sigmoid is monotonic) via two reduce_max passes.
  - Encoder GEMMs: x and W_enc cast to bf16 during DMA, x transposed per
    128x128 tile on TensorE, one PSUM accumulation over d per (k, b-tile,
    h-block); the per-k partial sums are scaled by coef and added into an
    SBUF accumulator (scalar_tensor_tensor); bias b_enc enters via a
    coef^T x b_enc matmul that initializes the accumulator.
No collectives: each core computes its own output shard independently.
"""
from contextlib import ExitStack

import numpy as np

import concourse.bass as bass
import concourse.tile as tile
from concourse import bacc, mybir
from concourse.bass_utils import run_bass_kernel_spmd
from concourse.masks import make_identity

N_CORES = 8
B, K, D, H, CTX, RH = 4096, 4, 1024, 1024, 256, 64
RH2 = RH // 2
BL = B // N_CORES  # 512 rows per core
NBT = BL // 128    # 4 batch tiles per core
DCH = D // 128     # 8 contraction chunks per modality
HB = 512           # h-block width (one PSUM bank)
NHB = H // HB      # 2 h-blocks
EPS = 1e-5
F32 = mybir.dt.float32
BF16 = mybir.dt.bfloat16
AF = mybir.ActivationFunctionType
OP = mybir.AluOpType
AX = mybir.AxisListType


def _build():
    nc = bacc.Bacc("TRN2", target_bir_lowering=False, debug=False,
                   num_devices=N_CORES)

    def din(name, shape):
        return nc.dram_tensor(name, shape, F32, kind="ExternalInput").ap()

    ctx_e = din("context", [BL, CTX])
    x_e = din("x", [K, BL, D])
    gum_e = din("gumbel", [BL, K])
    W1_e = din("W1", [CTX, RH])
    b1_e = din("b1", [1, RH])
    gln_e = din("g_ln", [1, RH])
    bln_e = din("beta_ln", [1, RH])
    W2_e = din("W2", [RH, RH2])
    b2_e = din("b2", [1, RH2])
    W3_e = din("W3", [RH2, K])
    b3_e = din("b3", [1, K])
    pr_e = din("prior", [1, K])
    We_e = din("W_enc", [K, D, H])
    be_e = din("b_enc", [K, H])
    fw_e = din("fusion_w", [1, K])
    out_e = nc.dram_tensor("out", [BL, H], F32, kind="ExternalOutput").ap()

    with tile.TileContext(nc) as tc, ExitStack() as st:
        singles = st.enter_context(tc.tile_pool(name="singles", bufs=1))
        rt = st.enter_context(tc.tile_pool(name="rt", bufs=2))
        wkp = st.enter_context(tc.tile_pool(name="wkp", bufs=16))
        xtp = st.enter_context(tc.tile_pool(name="xtp", bufs=16))
        xsp = st.enter_context(tc.tile_pool(name="xsp", bufs=16))
        psmm = st.enter_context(tc.tile_pool(name="psmm", bufs=2, space="PSUM"))
        pst = st.enter_context(tc.tile_pool(name="pst", bufs=4, space="PSUM"))

        # ---- constants ----
        ident = singles.tile([128, 128], F32)
        make_identity(nc, ident[:])
        eps64 = singles.tile([RH, 1], F32)
        nc.vector.memset(eps64[:], EPS)
        ones_row = singles.tile([1, BL], F32)
        nc.vector.memset(ones_row[:], 1.0)
        inv64_col = singles.tile([RH, 1], F32)
        nc.vector.memset(inv64_col[:], 1.0 / RH)
        eps1 = singles.tile([1, 1], F32)
        nc.vector.memset(eps1[:], EPS)

        # ---- small input DMAs ----
        ctx_sb = singles.tile([128, NBT, CTX], F32)
        nc.sync.dma_start(out=ctx_sb[:], in_=ctx_e.rearrange("(t p) c -> p t c", p=128))
        gum_sb = singles.tile([128, NBT, K], F32)
        nc.sync.dma_start(out=gum_sb[:], in_=gum_e.rearrange("(t p) k -> p t k", p=128))
        W1_sb = singles.tile([128, 2, RH], F32)
        nc.sync.dma_start(out=W1_sb[:], in_=W1_e.rearrange("(c p) r -> p c r", p=128))
        W2_sb = singles.tile([RH, RH2], F32)
        nc.sync.dma_start(out=W2_sb[:], in_=W2_e[:])
        W3_sb = singles.tile([RH2, K], F32)
        nc.sync.dma_start(out=W3_sb[:], in_=W3_e[:])
        b1_sb = singles.tile([RH, 1], F32)
        nc.sync.dma_start(out=b1_sb[:], in_=b1_e.rearrange("a r -> r a"))
        b2_sb = singles.tile([RH2, 1], F32)
        nc.sync.dma_start(out=b2_sb[:], in_=b2_e.rearrange("a r -> r a"))
        b3_sb = singles.tile([K, 1], F32)
        nc.sync.dma_start(out=b3_sb[:], in_=b3_e.rearrange("a r -> r a"))
        pr_sb = singles.tile([K, 1], F32)
        nc.sync.dma_start(out=pr_sb[:], in_=pr_e.rearrange("a r -> r a"))
        benc_sb = singles.tile([K, H], BF16)
        nc.gpsimd.dma_start(out=benc_sb[:], in_=be_e[:])
        # g_ln/beta_ln as per-partition columns [64, 1]
        gln_sb = singles.tile([RH, 1], F32)
        nc.sync.dma_start(out=gln_sb[:], in_=gln_e.rearrange("a r -> r a"))
        bln_sb = singles.tile([RH, 1], F32)
        nc.sync.dma_start(out=bln_sb[:], in_=bln_e.rearrange("a r -> r a"))
        fw_bc = singles.tile([128, K], F32)
        nc.gpsimd.dma_start(out=fw_bc[:], in_=fw_e.to_broadcast([128, K]))

        b3p = singles.tile([K, 1], F32)
        nc.vector.tensor_tensor(out=b3p[:], in0=b3_sb[:], in1=pr_sb[:], op=OP.add)

        # softmax(fusion_w) replicated per partition -> w4 [128, K]
        fex = singles.tile([128, K], F32)
        nc.scalar.activation(out=fex[:], in_=fw_bc[:], func=AF.Exp)
        fsum = singles.tile([128, 1], F32)
        nc.vector.reduce_sum(out=fsum[:], in_=fex[:], axis=AX.X)
        frec = singles.tile([128, 1], F32)
        nc.vector.reciprocal(out=frec[:], in_=fsum[:])
        w4 = singles.tile([128, K], F32)
        nc.vector.tensor_scalar_mul(out=w4[:], in0=fex[:], scalar1=frec[:])

        acc = singles.tile([128, NBT, H], F32)
        coef = singles.tile([128, NBT, K], F32)
        coefT = singles.tile([K, NBT, 128], BF16)

        # ---- big input DMAs, interleaved so nothing head-of-line blocks ----
        xts = [[None] * NBT for _ in range(K)]
        wks = [None] * K

        def emit_x_dmas(k):
            for bt in range(NBT):
                xt = xtp.tile([128, D], BF16, tag="xt")
                nc.gpsimd.dma_start(out=xt[:], in_=x_e[k, bt * 128:(bt + 1) * 128, :])
                xts[k][bt] = xt

        def emit_w_dma(k):
            quarters = []
            wv = We_e[k].rearrange("(c p) h -> p c h", p=128)
            for qq in range(4):
                wk = wkp.tile([128, DCH // 4, H], BF16, tag="wk")
                nc.gpsimd.dma_start(out=wk[:], in_=wv[:, qq * 2:(qq + 1) * 2, :])
                quarters.append(wk)
            wks[k] = quarters

        xsTs = {}

        def emit_transposes(k):
            """DMA-xbar transposes of x[k] tiles into xsT (bf16), per b-tile."""
            res = []
            for bt in range(NBT):
                xsT = xsp.tile([128, DCH, 128], BF16, tag="xsT")
                nc.sync.dma_start_transpose(out=xsT[:], in_=xts[k][bt][:])
                res.append(xsT)
            xsTs[k] = res

        for k in range(K):
            emit_x_dmas(k)
            emit_w_dma(k)
            emit_transposes(k)

        # ---- router part 1: ctx^T, hT = (ctx @ W1 + b1)^T, LN column sums ----
        ctxT = singles.tile([128, 2, BL], F32)
        for bt in range(NBT):
            for c in range(2):
                ps = pst.tile([128, 128], F32, tag="ps")
                nc.tensor.transpose(out=ps[:], in_=ctx_sb[:, bt, c * 128:(c + 1) * 128],
                                    identity=ident[:])
                nc.vector.tensor_copy(out=ctxT[:, c, bt * 128:(bt + 1) * 128], in_=ps[:])

        hps = pst.tile([RH, BL], F32, tag="ps")
        nc.tensor.matmul(out=hps[:], lhsT=W1_sb[:, 0, :], rhs=ctxT[:, 0, :],
                         start=True, stop=False)
        nc.tensor.matmul(out=hps[:], lhsT=W1_sb[:, 1, :], rhs=ctxT[:, 1, :],
                         start=False, stop=True)
        hT_raw = rt.tile([RH, BL], F32, tag="hT_raw")
        nc.vector.tensor_scalar_add(out=hT_raw[:], in0=hps[:], scalar1=b1_sb[:])
        hsq = rt.tile([RH, BL], F32, tag="hsq")
        nc.vector.tensor_tensor(out=hsq[:], in0=hT_raw[:], in1=hT_raw[:], op=OP.mult)
        mups = pst.tile([1, BL], F32, tag="ps")
        nc.tensor.matmul(out=mups[:], lhsT=inv64_col[:], rhs=hT_raw[:],
                         start=True, stop=True)
        msps = pst.tile([1, BL], F32, tag="ps")
        nc.tensor.matmul(out=msps[:], lhsT=inv64_col[:], rhs=hsq[:],
                         start=True, stop=True)
        mu1 = rt.tile([1, BL], F32, tag="mu1")
        nc.vector.tensor_copy(out=mu1[:], in_=mups[:])
        ms1 = rt.tile([1, BL], F32, tag="ms1")
        nc.vector.tensor_copy(out=ms1[:], in_=msps[:])

        # ---- keep PE busy with k=0 transposes while DVE/ACT do LN math ----
        emit_transposes(0)

        # ---- router part 2: var, rstd, broadcast, normalize, GEMM2/3 ----
        musq = rt.tile([1, BL], F32, tag="musq")
        nc.vector.tensor_tensor(out=musq[:], in0=mu1[:], in1=mu1[:], op=OP.mult)
        var1 = rt.tile([1, BL], F32, tag="var1")
        nc.vector.tensor_tensor(out=var1[:], in0=ms1[:], in1=musq[:], op=OP.subtract)
        rstd1 = rt.tile([1, BL], F32, tag="rstd1")
        nc.scalar.activation(out=rstd1[:], in_=var1[:], func=AF.Sqrt, bias=eps1[:])
        nc.vector.reciprocal(out=rstd1[:], in_=rstd1[:])

        mubc = pst.tile([RH, BL], F32, tag="ps")
        nc.tensor.matmul(out=mubc[:], lhsT=ones_row[:, 0:RH], rhs=mu1[:],
                         start=True, stop=True)
        rsbc = pst.tile([RH, BL], F32, tag="ps")
        nc.tensor.matmul(out=rsbc[:], lhsT=ones_row[:, 0:RH], rhs=rstd1[:],
                         start=True, stop=True)
        hn = rt.tile([RH, BL], F32, tag="hn")
        nc.vector.tensor_tensor(out=hn[:], in0=hT_raw[:], in1=mubc[:], op=OP.subtract)
        nc.vector.tensor_tensor(out=hn[:], in0=hn[:], in1=rsbc[:], op=OP.mult)
        nc.vector.tensor_scalar(out=hn[:], in0=hn[:], scalar1=gln_sb[:],
                                scalar2=bln_sb[:], op0=OP.mult, op1=OP.add)
        nc.vector.tensor_single_scalar(out=hn[:], in_=hn[:], scalar=0.0, op=OP.max)

        ps3 = pst.tile([RH2, BL], F32, tag="ps")
        nc.tensor.matmul(out=ps3[:], lhsT=W2_sb[:], rhs=hn[:], start=True, stop=True)
        h2T = rt.tile([RH2, BL], F32, tag="h2T")
        nc.vector.tensor_scalar(out=h2T[:], in0=ps3[:], scalar1=b2_sb[:],
                                scalar2=0.0, op0=OP.add, op1=OP.max)

        ps4 = pst.tile([K, BL], F32, tag="ps")
        nc.tensor.matmul(out=ps4[:], lhsT=W3_sb[:], rhs=h2T[:], start=True, stop=True)
        lgT = rt.tile([K, BL], F32, tag="lgT")
        nc.vector.tensor_scalar_add(out=lgT[:], in0=ps4[:], scalar1=b3p[:])

        # logits back to [b, K] per b-tile
        lg = singles.tile([128, NBT, K], F32)
        for bt in range(NBT):
            ps5 = pst.tile([128, K], F32, tag="ps")
            nc.tensor.transpose(out=ps5[:], in_=lgT[:, bt * 128:(bt + 1) * 128],
                                identity=ident[0:K, 0:K])
            nc.vector.tensor_copy(out=lg[:, bt, :], in_=ps5[:])

        # ---- mask pipeline, batched over b-tiles ([128, NBT, *] ops) ----
        # top-2 of 4 via minimax network (on logits; sigmoid is monotonic)
        s_all = rt.tile([128, NBT, K], F32, tag="s_all")
        nc.vector.tensor_tensor(out=s_all[:], in0=lg[:], in1=gum_sb[:], op=OP.add)
        soft_all = rt.tile([128, NBT, K], F32, tag="soft_all")
        nc.scalar.activation(out=soft_all[:], in_=s_all[:], func=AF.Sigmoid)

        a, b = lg[:, :, 0:1], lg[:, :, 1:2]
        c_, d_ = lg[:, :, 2:3], lg[:, :, 3:4]
        mab = rt.tile([128, NBT, 1], F32, tag="mab")
        nc.vector.tensor_tensor(out=mab[:], in0=a, in1=b, op=OP.max)
        mcd = rt.tile([128, NBT, 1], F32, tag="mcd")
        nc.vector.tensor_tensor(out=mcd[:], in0=c_, in1=d_, op=OP.max)
        nab = rt.tile([128, NBT, 1], F32, tag="nab")
        nc.vector.tensor_tensor(out=nab[:], in0=a, in1=b, op=OP.min)
        ncd = rt.tile([128, NBT, 1], F32, tag="ncd")
        nc.vector.tensor_tensor(out=ncd[:], in0=c_, in1=d_, op=OP.min)
        mmm = rt.tile([128, NBT, 1], F32, tag="mmm")
        nc.vector.tensor_tensor(out=mmm[:], in0=mab[:], in1=mcd[:], op=OP.min)
        m2a = rt.tile([128, NBT, 1], F32, tag="m2a")
        nc.vector.tensor_tensor(out=m2a[:], in0=nab[:], in1=ncd[:], op=OP.max)
        m2b = rt.tile([128, NBT, 1], F32, tag="m2b")
        nc.vector.tensor_tensor(out=m2b[:], in0=m2a[:], in1=mmm[:], op=OP.max)

        mnm = rt.tile([128, NBT, K], F32, tag="mnm")
        for kk in range(K):
            nc.vector.tensor_tensor(out=mnm[:, :, kk:kk + 1], in0=lg[:, :, kk:kk + 1],
                                    in1=m2b[:], op=OP.is_ge)
        msk = rt.tile([128, NBT, K], F32, tag="msk")
        nc.vector.tensor_tensor(out=msk[:], in0=soft_all[:], in1=mnm[:], op=OP.max)
        hm = rt.tile([128, NBT, K], F32, tag="hm")
        nc.vector.scalar_tensor_tensor(out=hm[:], in0=msk[:], scalar=0.5,
                                       in1=msk[:], op0=OP.is_gt, op1=OP.mult)
        for kk in range(K):
            nc.vector.tensor_scalar_mul(out=coef[:, :, kk:kk + 1],
                                        in0=hm[:, :, kk:kk + 1],
                                        scalar1=w4[:, kk:kk + 1])

        # ---- main encoder GEMMs, k-outer ----
        def emit_mm_block(k):
            for bt in range(NBT):
                xsT = xsTs[k][bt]
                pm = psmm.tile([128, NHB, HB], F32, tag="mm")
                for hb in range(NHB):
                    for c in range(DCH):
                        nc.tensor.matmul(out=pm[:, hb, :],
                                         lhsT=xsT[:, c, :],
                                         rhs=wks[k][c // 2][:, c % 2,
                                                           hb * HB:(hb + 1) * HB],
                                         start=(c == 0),
                                         stop=(c == DCH - 1))
                if k == 0:
                    nc.vector.tensor_scalar_mul(
                        out=acc[:, bt, :], in0=pm[:].rearrange("p a b -> p (a b)"),
                        scalar1=coef[:, bt, 0:1])
                else:
                    nc.vector.scalar_tensor_tensor(
                        out=acc[:, bt, :], in0=pm[:].rearrange("p a b -> p (a b)"),
                        scalar=coef[:, bt, k:k + 1], in1=acc[:, bt, :],
                        op0=OP.mult, op1=OP.add)
                if k == K - 1:
                    nc.sync.dma_start(out=out_e[bt * 128:(bt + 1) * 128, :],
                                      in_=acc[:, bt, :])

        emit_mm_block(0)

        # coef^T + b_enc bias, added after k=0 (PE busy while mask ran on DVE)
        for bt in range(NBT):
            ps6 = pst.tile([K, 128], F32, tag="ps")
            nc.tensor.transpose(out=ps6[:], in_=coef[:, bt, :], identity=ident[:])
            nc.vector.tensor_copy(out=coefT[:, bt, :], in_=ps6[:])
        for bt in range(NBT):
            for hb in range(NHB):
                hsl = slice(hb * HB, (hb + 1) * HB)
                pmb = psmm.tile([128, HB], F32, tag="mm")
                nc.tensor.matmul(out=pmb[:], lhsT=coefT[:, bt, :],
                                 rhs=benc_sb[:, hsl], start=True, stop=True)
                nc.vector.tensor_tensor(out=acc[:, bt, hsl], in0=acc[:, bt, hsl],
                                        in1=pmb[:], op=OP.add)

        for k in range(1, K):
            emit_mm_block(k)

    nc.compile()
    return nc


_NC = None


def _get_nc():
    global _NC
    if _NC is None:
        _NC = _build()
    return _NC


def kernel(**inputs):
    nc = _get_nc()
    f = {k: np.ascontiguousarray(np.asarray(v, dtype=np.float32))
         for k, v in inputs.items()}
    shared = {
        "W1": f["W1"],
        "b1": f["b1"].reshape(1, RH),
        "g_ln": f["g_ln"].reshape(1, RH),
        "beta_ln": f["beta_ln"].reshape(1, RH),
        "W2": f["W2"],
        "b2": f["b2"].reshape(1, RH2),
        "W3": f["W3"],
        "b3": f["b3"].reshape(1, K),
        "prior": f["prior"].reshape(1, K),
        "W_enc": f["W_enc"],
        "b_enc": f["b_enc"],
        "fusion_w": f["fusion_w"].reshape(1, K),
    }
    in_maps = []
    for i in range(N_CORES):
        sl = slice(i * BL, (i + 1) * BL)
        m = dict(shared)
        m["context"] = np.ascontiguousarray(f["context"][sl])
        m["x"] = np.ascontiguousarray(f["x"][:, sl, :])
        m["gumbel"] = np.ascontiguousarray(f["gumbel"][sl])
        in_maps.append(m)
    res = run_bass_kernel_spmd(nc, in_maps, core_ids=list(range(N_CORES)))
    return np.concatenate([res.results[i]["out"] for i in range(N_CORES)], axis=0)
